# revision 9
# baseline (speedup 1.0000x reference)
"""MoE-LoRA Linear kernel for 8 Trainium2 NeuronCores.

Sharding: core c -> (batch b = c//2, out-feature half = c%2).
Each core computes out[b, :, half] = x[b] @ W_half.T + b_half
                                   + SCALING * router-weighted LoRA.

All matmul operands are bf16 (fp32 accumulation in PSUM); x is fully
resident in SBUF so W is streamed exactly once.

Device layout (per core):
  xs   [128, 32, 2048] bf16  x[b].T tiled d=(dt*128+p), streamed in 32 chunks
  wt   [128, 32, 128]  bf16  W o-tile (streamed, 3 bufs)
  arw  [128, 32, 128]  bf16  stationary cols: 0-63 lora_A (er), 64-127
                             router_W replicated 8x over r
  bta  [65, 2048]      bf16  rows 0-63: lora_B[half] as [er, o]; row 64: b_base
  haug [65, 2048]      bf16  router-scaled h; row 64 = ones (bias)
  out  [2048, 2048]    f32   result transposed: [o, t]

The h matmul (stationary=arw) yields h rows 0:63 AND router logit
partials rows 64:127 in one pass; logits = DVE row-sum over t.  Softmax
over the 64 replicated logits gives w/8 per row; scaling by 16 folds in
SCALING=2.  Each o-tile accumulates 32 d-matmuls + one K=65 lora/bias
matmul (rows of bta, ones-row of haug) into the same PSUM group.
"""
import sys

sys.path.insert(0, "/opt/trn_rl_repo")

import numpy as np
import ml_dtypes

import concourse.bass as bass
import concourse.mybir as mybir
import concourse.tile as tile
from concourse import bacc, bass_isa
from concourse.bass_utils import run_bass_kernel_spmd

F32 = mybir.dt.float32
BF16 = mybir.dt.bfloat16
NPBF = ml_dtypes.bfloat16

D, T, O_SH, E, R = 4096, 2048, 2048, 8, 8
ER = E * R  # 64
DT = D // 128  # 32 d-tiles
OT = O_SH // 128  # 16 o-tiles
NT4 = T // 512  # 4 psum-width chunks
ROUTER_TEMP = 1.0
SCALING = 16.0 / 8.0

_nc_cache = []


def build():
    nc = bacc.Bacc(None, target_bir_lowering=False)
    XS = nc.dram_tensor("XS", [128, DT * T], BF16, kind="ExternalInput")
    WT = nc.dram_tensor("WT", [OT * 128, DT * 128], BF16, kind="ExternalInput")
    ARW = nc.dram_tensor("ARW", [128, DT * 128], BF16, kind="ExternalInput")
    BTA = nc.dram_tensor("BTA", [ER + 1, O_SH], BF16, kind="ExternalInput")
    RB = nc.dram_tensor("RB", [ER], F32, kind="ExternalInput")
    out = nc.dram_tensor("out", [O_SH, T], F32, kind="ExternalOutput")

    with tile.TileContext(nc) as tc:
        with (
            tc.tile_pool(name="xp", bufs=1) as xp,
            tc.tile_pool(name="wp", bufs=3) as wp,
            tc.tile_pool(name="sg", bufs=1) as sg,
            tc.tile_pool(name="ev", bufs=4) as evp,
            tc.tile_pool(name="ps", bufs=8, space="PSUM") as psp,
        ):
            arw = sg.tile([128, DT, 128], BF16)
            nc.sync.dma_start(arw[:], ARW[:].rearrange("p (dt c) -> p dt c", c=128))
            bta = sg.tile([ER + 1, O_SH], BF16)
            nc.sync.dma_start(bta[:], BTA[:])
            rb = sg.tile([ER, 1], F32)
            nc.sync.dma_start(rb[:], RB[:, None])
            haug = sg.tile([ER + 1, T], BF16)
            nc.vector.memset(haug[ER : ER + 1, :], 1.0)

            # x chunks go on the scalar HWDGE ring, issued up-front with no
            # slot waits, so nothing can starve the PE's x stream.  W/out
            # traffic shares the sync ring.
            xs = xp.tile([128, DT, T], BF16)
            for d in range(DT):
                nc.scalar.dma_start(xs[:, d, :], XS[:, d * T : (d + 1) * T])

            def load_w(o):
                wt = wp.tile([128, DT, 128], BF16, tag="wt", name=f"wt{o}")
                # o=1,2 ride the scalar ring BEHIND the x chunks so their
                # prefetch can't steal HBM bandwidth from the x stream; they
                # arrive right as the load phase ends, which is when o1 starts.
                eng = nc.scalar if o in (1, 2) else nc.sync
                eng.dma_start(
                    wt[:],
                    WT[o * 128 : (o + 1) * 128, :].rearrange(
                        "p (dt c) -> p dt c", c=128
                    ),
                )
                return wt

            # h (+ router logit partials) and o-tile 0, paced by the x stream
            wt0 = load_w(0)
            hps = [psp.tile([128, 512], F32, tag="ps", name=f"h{t}") for t in range(NT4)]
            ps0 = [psp.tile([128, 512], F32, tag="ps", name=f"p0_{t}") for t in range(NT4)]
            for d in range(DT):
                for t4 in range(NT4):
                    nc.tensor.matmul(
                        hps[t4][:],
                        arw[:, d, :],
                        xs[:, d, t4 * 512 : (t4 + 1) * 512],
                        start=(d == 0),
                        stop=(d == DT - 1),
                    )
                for t4 in range(NT4):
                    nc.tensor.matmul(
                        ps0[t4][:],
                        wt0[:, d, :],
                        xs[:, d, t4 * 512 : (t4 + 1) * 512],
                        start=(d == 0),
                        stop=False,
                    )

            # Evacuate h PSUM to SBUF immediately: frees the 4 banks for o1
            # before the softmax chain runs.
            hsb = sg.tile([128, NT4, 512], F32)
            for t4 in range(NT4):
                nc.vector.tensor_copy(hsb[:, t4, :], hps[t4][:])

            # router: logits = rowsum_t(h[64:128]) / T + rb; softmax over the
            # 64 replicated rows (sum = 8*S) -> w/8; *16 folds SCALING=2.
            lgq = sg.tile([ER, NT4], F32)
            for t4 in range(NT4):
                nc.vector.reduce_sum(
                    lgq[:, t4 : t4 + 1], hsb[ER:128, t4, :], axis=mybir.AxisListType.X
                )
            lg = sg.tile([ER, 1], F32)
            nc.vector.reduce_sum(lg[:], lgq[:], axis=mybir.AxisListType.X)
            nc.scalar.activation(
                lg[:], lg[:], mybir.ActivationFunctionType.Copy,
                scale=1.0 / (T * ROUTER_TEMP),
            )
            nc.vector.tensor_tensor(lg[:], lg[:], rb[:], mybir.AluOpType.add)
            mx = sg.tile([ER, 1], F32)
            nc.gpsimd.partition_all_reduce(
                mx[:], lg[:], channels=ER, reduce_op=bass_isa.ReduceOp.max
            )
            nc.vector.tensor_tensor(lg[:], lg[:], mx[:], mybir.AluOpType.subtract)
            nc.scalar.activation(lg[:], lg[:], mybir.ActivationFunctionType.Exp)
            sm = sg.tile([ER, 1], F32)
            nc.gpsimd.partition_all_reduce(
                sm[:], lg[:], channels=ER, reduce_op=bass_isa.ReduceOp.add
            )
            rcp = sg.tile([ER, 1], F32)
            nc.vector.reciprocal(rcp[:], sm[:])
            w64 = sg.tile([ER, 1], F32)
            nc.vector.tensor_tensor(w64[:], lg[:], rcp[:], mybir.AluOpType.mult)
            nc.vector.tensor_scalar_mul(w64[:], w64[:], 8.0 * SCALING)
            for t4 in range(NT4):
                nc.vector.tensor_tensor(
                    haug[0:ER, t4 * 512 : (t4 + 1) * 512],
                    hsb[0:ER, t4, :],
                    w64[:].to_broadcast([ER, 512]),
                    mybir.AluOpType.mult,
                )

            def lora_and_evac(o, pso):
                for t4 in range(NT4):
                    nc.tensor.matmul(
                        pso[t4][:],
                        bta[:, o * 128 : (o + 1) * 128],
                        haug[:, t4 * 512 : (t4 + 1) * 512],
                        start=False,
                        stop=True,
                    )
                for t4 in range(NT4):
                    ev = evp.tile([128, 512], F32, tag="ev")
                    nc.vector.tensor_copy(ev[:], pso[t4][:])
                    nc.sync.dma_start(
                        out[o * 128 : (o + 1) * 128, t4 * 512 : (t4 + 1) * 512], ev[:]
                    )

            prev = (0, ps0)
            for o in range(1, OT):
                wt = load_w(o)
                pso = [
                    psp.tile([128, 512], F32, tag="ps", name=f"p{o}_{t}")
                    for t in range(NT4)
                ]
                for d in range(DT):
                    for t4 in range(NT4):
                        nc.tensor.matmul(
                            pso[t4][:],
                            wt[:, d, :],
                            xs[:, d, t4 * 512 : (t4 + 1) * 512],
                            start=(d == 0),
                            stop=False,
                        )
                lora_and_evac(*prev)
                prev = (o, pso)
            lora_and_evac(*prev)
    nc.compile()
    return nc


def _get_nc():
    if not _nc_cache:
        _nc_cache.append(build())
    return _nc_cache[0]


def _tile_dmajor(a_dT):
    """[D, C] (d-major rows) -> [128, DT*C] with d = dt*128 + p."""
    Dd, C = a_dT.shape
    return np.ascontiguousarray(
        a_dT.reshape(Dd // 128, 128, C).swapaxes(0, 1)
    ).reshape(128, (Dd // 128) * C)


def kernel(x, W_base, b_base, lora_A, lora_B, router_W, router_b):
    x = np.asarray(x, dtype=np.float32)
    W_base = np.asarray(W_base, dtype=np.float32)
    b_base = np.asarray(b_base, dtype=np.float32)
    lora_A = np.asarray(lora_A, dtype=np.float32)
    lora_B = np.asarray(lora_B, dtype=np.float32)
    router_W = np.asarray(router_W, dtype=np.float32)
    router_b = np.asarray(router_b, dtype=np.float32)

    B, S, D_ = x.shape
    O = W_base.shape[0]

    xs_list = []
    for b in range(B):
        xt = np.ascontiguousarray(x[b].astype(NPBF).T)  # [D, T]
        xs_list.append(_tile_dmajor(xt))

    wt_list = []
    for hh in range(2):
        Wh = np.ascontiguousarray(W_base[hh * O_SH : (hh + 1) * O_SH].astype(NPBF).T)
        # [D, O_SH] -> [OT, 128, DT*128]
        w4 = Wh.reshape(DT, 128, OT, 128).transpose(2, 1, 0, 3)
        wt_list.append(np.ascontiguousarray(w4).reshape(OT * 128, DT * 128))

    At = lora_A.reshape(ER, D_)  # [er, d]
    rw64 = np.repeat(router_W, R, axis=0)  # [er, d]
    arw_h = _tile_dmajor(
        np.ascontiguousarray(np.concatenate([At, rw64], axis=0).astype(NPBF).T)
    )

    bta_list = []
    for hh in range(2):
        osl = slice(hh * O_SH, (hh + 1) * O_SH)
        Bt = lora_B[:, osl, :].transpose(0, 2, 1).reshape(ER, O_SH)
        bta_list.append(
            np.ascontiguousarray(
                np.concatenate([Bt, b_base[osl][None, :]], axis=0).astype(NPBF)
            )
        )
    rb64 = np.ascontiguousarray(np.repeat(router_b, R).astype(np.float32))

    in_maps = []
    for c in range(8):
        b, hh = c // 2, c % 2
        in_maps.append(
            {
                "XS": xs_list[b],
                "WT": wt_list[hh],
                "ARW": arw_h,
                "BTA": bta_list[hh],
                "RB": rb64,
            }
        )

    global _last_in_maps
    _last_in_maps = in_maps
    nc = _get_nc()
    res = run_bass_kernel_spmd(nc, in_maps, core_ids=list(range(8)))
    out = np.empty((B, S, O), dtype=np.float32)
    for c in range(8):
        b, hh = c // 2, c % 2
        out[b, :, hh * O_SH : (hh + 1) * O_SH] = res.results[c]["out"].T
    return out


# revision 10
# speedup vs baseline: 1.1733x; 1.1733x over previous
"""MoE-LoRA Linear kernel for 8 Trainium2 NeuronCores.

Sharding: core c -> (batch b = c//2, out-feature half = c%2).
Each core computes out[b, :, half] = x[b] @ W_half.T + b_half
                                   + SCALING * router-weighted LoRA.

All matmul operands are bf16 (fp32 accumulation in PSUM); x is fully
resident in SBUF so W is streamed exactly once.

Device layout (per core):
  xs   [128, 32, 2048] bf16  x[b].T tiled d=(dt*128+p), streamed in 32 chunks
  wt   [128, 32, 128]  bf16  W o-tile (streamed, 3 bufs)
  arw  [128, 32, 128]  bf16  stationary cols: 0-63 lora_A (er), 64-127
                             router_W replicated 8x over r
  bta  [65, 2048]      bf16  rows 0-63: lora_B[half] as [er, o]; row 64: b_base
  haug [65, 2048]      bf16  router-scaled h; row 64 = ones (bias)
  out  [2048, 2048]    f32   result transposed: [o, t]

The h matmul (stationary=arw) yields h rows 0:63 AND router logit
partials rows 64:127 in one pass; logits = DVE row-sum over t.  Softmax
over the 64 replicated logits gives w/8 per row; scaling by 16 folds in
SCALING=2.  Each o-tile accumulates 32 d-matmuls + one K=65 lora/bias
matmul (rows of bta, ones-row of haug) into the same PSUM group.
"""
import sys

sys.path.insert(0, "/opt/trn_rl_repo")

import numpy as np
import ml_dtypes

import concourse.bass as bass
import concourse.mybir as mybir
import concourse.tile as tile
from concourse import bacc, bass_isa
from concourse.bass_utils import run_bass_kernel_spmd

F32 = mybir.dt.float32
BF16 = mybir.dt.bfloat16
NPBF = ml_dtypes.bfloat16

D, T, O_SH, E, R = 4096, 2048, 2048, 8, 8
ER = E * R  # 64
DT = D // 128  # 32 d-tiles
OT = O_SH // 128  # 16 o-tiles
NT4 = T // 512  # 4 psum-width chunks
ROUTER_TEMP = 1.0
SCALING = 16.0 / 8.0

_nc_cache = []


def build():
    nc = bacc.Bacc(None, target_bir_lowering=False)
    XS = nc.dram_tensor("XS", [128, DT * T], BF16, kind="ExternalInput")
    WT = nc.dram_tensor("WT", [OT * 128, DT * 128], BF16, kind="ExternalInput")
    ARW = nc.dram_tensor("ARW", [128, DT * 128], BF16, kind="ExternalInput")
    BTA = nc.dram_tensor("BTA", [ER + 1, O_SH], BF16, kind="ExternalInput")
    RB = nc.dram_tensor("RB", [ER], F32, kind="ExternalInput")
    out = nc.dram_tensor("out", [O_SH, T], F32, kind="ExternalOutput")

    with tile.TileContext(nc) as tc:
        with (
            tc.tile_pool(name="xp", bufs=1) as xp,
            tc.tile_pool(name="wp", bufs=3) as wp,
            tc.tile_pool(name="sg", bufs=1) as sg,
            tc.tile_pool(name="ev", bufs=6) as evp,
            tc.tile_pool(name="ps", bufs=8, space="PSUM") as psp,
        ):
            arw = sg.tile([128, DT, 128], BF16)
            nc.sync.dma_start(arw[:], ARW[:].rearrange("p (dt c) -> p dt c", c=128))
            bta = sg.tile([ER + 1, O_SH], BF16)
            nc.sync.dma_start(bta[:], BTA[:])
            rb = sg.tile([ER, 1], F32)
            nc.sync.dma_start(rb[:], RB[:, None])
            haug = sg.tile([ER + 1, T], BF16)
            nc.vector.memset(haug[ER : ER + 1, :], 1.0)

            # x chunks go on the scalar HWDGE ring, issued up-front with no
            # slot waits, so nothing can starve the PE's x stream.  W/out
            # traffic shares the sync ring.
            xs = xp.tile([128, DT, T], BF16)
            for d in range(DT):
                nc.scalar.dma_start(xs[:, d, :], XS[:, d * T : (d + 1) * T])

            def load_w(o):
                wt = wp.tile([128, DT, 128], BF16, tag="wt", name=f"wt{o}")
                nc.sync.dma_start(
                    wt[:],
                    WT[o * 128 : (o + 1) * 128, :].rearrange(
                        "p (dt c) -> p dt c", c=128
                    ),
                )
                return wt

            # h (+ router logit partials) and o-tile 0, paced by the x stream
            wt0 = load_w(0)
            hps = [psp.tile([128, 512], F32, tag="ps", name=f"h{t}") for t in range(NT4)]
            ps0 = [psp.tile([128, 512], F32, tag="ps", name=f"p0_{t}") for t in range(NT4)]
            for d in range(DT):
                for t4 in range(NT4):
                    nc.tensor.matmul(
                        hps[t4][:],
                        arw[:, d, :],
                        xs[:, d, t4 * 512 : (t4 + 1) * 512],
                        start=(d == 0),
                        stop=(d == DT - 1),
                    )
                for t4 in range(NT4):
                    nc.tensor.matmul(
                        ps0[t4][:],
                        wt0[:, d, :],
                        xs[:, d, t4 * 512 : (t4 + 1) * 512],
                        start=(d == 0),
                        stop=False,
                    )

            # router: logits = rowsum_t(hps[64:128]) / T + rb; softmax over the
            # 64 replicated rows (sum = 8*S) -> w/8; *16 folds SCALING=2.
            lgq = sg.tile([ER, NT4], F32)
            for t4 in range(NT4):
                nc.vector.reduce_sum(
                    lgq[:, t4 : t4 + 1], hps[t4][ER:128, :], axis=mybir.AxisListType.X
                )
            lg = sg.tile([ER, 1], F32)
            nc.vector.reduce_sum(lg[:], lgq[:], axis=mybir.AxisListType.X)
            nc.scalar.activation(
                lg[:], lg[:], mybir.ActivationFunctionType.Copy,
                scale=1.0 / (T * ROUTER_TEMP),
            )
            nc.vector.tensor_tensor(lg[:], lg[:], rb[:], mybir.AluOpType.add)
            mx = sg.tile([ER, 1], F32)
            nc.gpsimd.partition_all_reduce(
                mx[:], lg[:], channels=ER, reduce_op=bass_isa.ReduceOp.max
            )
            nc.vector.tensor_tensor(lg[:], lg[:], mx[:], mybir.AluOpType.subtract)
            nc.scalar.activation(lg[:], lg[:], mybir.ActivationFunctionType.Exp)
            sm = sg.tile([ER, 1], F32)
            nc.gpsimd.partition_all_reduce(
                sm[:], lg[:], channels=ER, reduce_op=bass_isa.ReduceOp.add
            )
            rcp = sg.tile([ER, 1], F32)
            nc.vector.reciprocal(rcp[:], sm[:])
            w64 = sg.tile([ER, 1], F32)
            nc.vector.tensor_tensor(w64[:], lg[:], rcp[:], mybir.AluOpType.mult)
            nc.vector.tensor_scalar_mul(w64[:], w64[:], 8.0 * SCALING)
            for t4 in range(NT4):
                nc.vector.tensor_tensor(
                    haug[0:ER, t4 * 512 : (t4 + 1) * 512],
                    hps[t4][0:ER, :],
                    w64[:].to_broadcast([ER, 512]),
                    mybir.AluOpType.mult,
                )

            def lora_and_evac(o, pso):
                for t4 in range(NT4):
                    nc.tensor.matmul(
                        pso[t4][:],
                        bta[:, o * 128 : (o + 1) * 128],
                        haug[:, t4 * 512 : (t4 + 1) * 512],
                        start=False,
                        stop=True,
                    )
                for t4 in range(NT4):
                    ev = evp.tile([128, 512], F32, tag="ev")
                    nc.vector.tensor_copy(ev[:], pso[t4][:])
                    nc.sync.dma_start(
                        out[o * 128 : (o + 1) * 128, t4 * 512 : (t4 + 1) * 512], ev[:]
                    )

            prev = (0, ps0)
            for o in range(1, OT):
                wt = load_w(o)
                pso = [
                    psp.tile([128, 512], F32, tag="ps", name=f"p{o}_{t}")
                    for t in range(NT4)
                ]
                for d in range(DT):
                    for t4 in range(NT4):
                        nc.tensor.matmul(
                            pso[t4][:],
                            wt[:, d, :],
                            xs[:, d, t4 * 512 : (t4 + 1) * 512],
                            start=(d == 0),
                            stop=False,
                        )
                lora_and_evac(*prev)
                prev = (o, pso)
            lora_and_evac(*prev)
    nc.compile()
    return nc


def _get_nc():
    if not _nc_cache:
        _nc_cache.append(build())
    return _nc_cache[0]


def _tile_dmajor(a_dT):
    """[D, C] (d-major rows) -> [128, DT*C] with d = dt*128 + p."""
    Dd, C = a_dT.shape
    return np.ascontiguousarray(
        a_dT.reshape(Dd // 128, 128, C).swapaxes(0, 1)
    ).reshape(128, (Dd // 128) * C)


def kernel(x, W_base, b_base, lora_A, lora_B, router_W, router_b):
    x = np.asarray(x, dtype=np.float32)
    W_base = np.asarray(W_base, dtype=np.float32)
    b_base = np.asarray(b_base, dtype=np.float32)
    lora_A = np.asarray(lora_A, dtype=np.float32)
    lora_B = np.asarray(lora_B, dtype=np.float32)
    router_W = np.asarray(router_W, dtype=np.float32)
    router_b = np.asarray(router_b, dtype=np.float32)

    B, S, D_ = x.shape
    O = W_base.shape[0]

    xs_list = []
    for b in range(B):
        xt = np.ascontiguousarray(x[b].astype(NPBF).T)  # [D, T]
        xs_list.append(_tile_dmajor(xt))

    wt_list = []
    for hh in range(2):
        Wh = np.ascontiguousarray(W_base[hh * O_SH : (hh + 1) * O_SH].astype(NPBF).T)
        # [D, O_SH] -> [OT, 128, DT*128]
        w4 = Wh.reshape(DT, 128, OT, 128).transpose(2, 1, 0, 3)
        wt_list.append(np.ascontiguousarray(w4).reshape(OT * 128, DT * 128))

    At = lora_A.reshape(ER, D_)  # [er, d]
    rw64 = np.repeat(router_W, R, axis=0)  # [er, d]
    arw_h = _tile_dmajor(
        np.ascontiguousarray(np.concatenate([At, rw64], axis=0).astype(NPBF).T)
    )

    bta_list = []
    for hh in range(2):
        osl = slice(hh * O_SH, (hh + 1) * O_SH)
        Bt = lora_B[:, osl, :].transpose(0, 2, 1).reshape(ER, O_SH)
        bta_list.append(
            np.ascontiguousarray(
                np.concatenate([Bt, b_base[osl][None, :]], axis=0).astype(NPBF)
            )
        )
    rb64 = np.ascontiguousarray(np.repeat(router_b, R).astype(np.float32))

    in_maps = []
    for c in range(8):
        b, hh = c // 2, c % 2
        in_maps.append(
            {
                "XS": xs_list[b],
                "WT": wt_list[hh],
                "ARW": arw_h,
                "BTA": bta_list[hh],
                "RB": rb64,
            }
        )

    global _last_in_maps
    _last_in_maps = in_maps
    nc = _get_nc()
    res = run_bass_kernel_spmd(nc, in_maps, core_ids=list(range(8)))
    out = np.empty((B, S, O), dtype=np.float32)
    for c in range(8):
        b, hh = c // 2, c % 2
        out[b, :, hh * O_SH : (hh + 1) * O_SH] = res.results[c]["out"].T
    return out


# revision 15
# speedup vs baseline: 1.1777x; 1.0037x over previous
"""MoE-LoRA Linear kernel for 8 Trainium2 NeuronCores.

Sharding: core c -> (batch b = c//2, out-feature half = c%2).
Each core computes out[b, :, half] = x[b] @ W_half.T + b_half
                                   + SCALING * router-weighted LoRA.

All matmul operands are bf16 (fp32 accumulation in PSUM); x is fully
resident in SBUF so W is streamed exactly once.

Device layout (per core):
  xs   [128, 32, 2048] bf16  x[b].T tiled d=(dt*128+p), streamed in 32 chunks
  wt   [128, 32, 128]  bf16  W o-tile (streamed, 3 bufs)
  arw  [128, 32, 128]  bf16  stationary cols: 0-63 lora_A (er), 64-127
                             router_W replicated 8x over r
  bta  [65, 2048]      bf16  rows 0-63: lora_B[half] as [er, o]; row 64: b_base
  haug [65, 2048]      bf16  router-scaled h; row 64 = ones (bias)
  out  [2048, 2048]    f32   result transposed: [o, t]

The h matmul (stationary=arw) yields h rows 0:63 AND router logit
partials rows 64:127 in one pass; logits = DVE row-sum over t.  Softmax
over the 64 replicated logits gives w/8 per row; scaling by 16 folds in
SCALING=2.  Each o-tile accumulates 32 d-matmuls + one K=65 lora/bias
matmul (rows of bta, ones-row of haug) into the same PSUM group.
"""
import sys

sys.path.insert(0, "/opt/trn_rl_repo")

import numpy as np
import ml_dtypes

import concourse.bass as bass
import concourse.mybir as mybir
import concourse.tile as tile
from concourse import bacc, bass_isa
from concourse.bass import _add_dep_helper
from concourse.bass_utils import run_bass_kernel_spmd

F32 = mybir.dt.float32
BF16 = mybir.dt.bfloat16
NPBF = ml_dtypes.bfloat16

D, T, O_SH, E, R = 4096, 2048, 2048, 8, 8
ER = E * R  # 64
DT = D // 128  # 32 d-tiles
OT = O_SH // 128  # 16 o-tiles
NT4 = T // 512  # 4 psum-width chunks
ROUTER_TEMP = 1.0
SCALING = 16.0 / 8.0

_nc_cache = []


def build():
    nc = bacc.Bacc(None, target_bir_lowering=False)
    XS = nc.dram_tensor("XS", [128, DT * T], BF16, kind="ExternalInput")
    WT = nc.dram_tensor("WT", [OT * 128, DT * 128], BF16, kind="ExternalInput")
    ARW = nc.dram_tensor("ARW", [128, DT * 128], BF16, kind="ExternalInput")
    BTA = nc.dram_tensor("BTA", [ER + 1, O_SH], BF16, kind="ExternalInput")
    RB = nc.dram_tensor("RB", [ER], F32, kind="ExternalInput")
    out = nc.dram_tensor("out", [O_SH, T], F32, kind="ExternalOutput")

    with tile.TileContext(nc) as tc:
        with (
            tc.tile_pool(name="xp", bufs=1) as xp,
            tc.tile_pool(name="wp", bufs=3) as wp,
            tc.tile_pool(name="sg", bufs=1) as sg,
            tc.tile_pool(name="ev", bufs=6) as evp,
            tc.tile_pool(name="ps", bufs=8, space="PSUM") as psp,
        ):
            arw = sg.tile([128, DT, 128], BF16)
            nc.sync.dma_start(arw[:], ARW[:].rearrange("p (dt c) -> p dt c", c=128))
            bta = sg.tile([ER + 1, O_SH], BF16)
            bta_dma = nc.sync.dma_start(bta[:], BTA[:])
            rb = sg.tile([ER, 1], F32)
            nc.sync.dma_start(rb[:], RB[:, None])
            haug = sg.tile([ER + 1, T], BF16)
            nc.vector.memset(haug[ER : ER + 1, :], 1.0)

            # x chunks go on the scalar HWDGE ring, issued up-front with no
            # slot waits, so nothing can starve the PE's x stream.  W/out
            # traffic shares the sync ring.
            xs = xp.tile([128, DT, T], BF16)
            x_dmas = []
            for d in range(DT):
                x_dmas.append(
                    nc.scalar.dma_start(xs[:, d, :], XS[:, d * T : (d + 1) * T])
                )

            w_delay = {1: 15, 2: 23}

            def load_w(o):
                wt = wp.tile([128, DT, 128], BF16, tag="wt", name=f"wt{o}")
                dma = nc.sync.dma_start(
                    wt[:],
                    WT[o * 128 : (o + 1) * 128, :].rearrange(
                        "p (dt c) -> p dt c", c=128
                    ),
                )
                # Hold the o=1,2 prefetches (and bta) until the x stream is
                # mostly in: during the first ~25us every byte of HBM bandwidth
                # must feed x, or the PE starves at the start of the load loop.
                if o in w_delay:
                    _add_dep_helper(
                        dma.ins, x_dmas[w_delay[o]].ins, sync=True,
                        reason="delay W prefetch behind x stream",
                    )
                return wt

            _add_dep_helper(
                bta_dma.ins, x_dmas[19].ins, sync=True,
                reason="delay bta behind x stream",
            )

            # h (+ router logit partials) and o-tile 0, paced by the x stream
            wt0 = load_w(0)
            hps = [psp.tile([128, 512], F32, tag="ps", name=f"h{t}") for t in range(NT4)]
            ps0 = [psp.tile([128, 512], F32, tag="ps", name=f"p0_{t}") for t in range(NT4)]
            for d in range(DT):
                for t4 in range(NT4):
                    nc.tensor.matmul(
                        hps[t4][:],
                        arw[:, d, :],
                        xs[:, d, t4 * 512 : (t4 + 1) * 512],
                        start=(d == 0),
                        stop=(d == DT - 1),
                    )
                for t4 in range(NT4):
                    nc.tensor.matmul(
                        ps0[t4][:],
                        wt0[:, d, :],
                        xs[:, d, t4 * 512 : (t4 + 1) * 512],
                        start=(d == 0),
                        stop=False,
                    )

            # router: logits = rowsum_t(hps[64:128]) / T + rb; softmax over the
            # 64 replicated rows (sum = 8*S) -> w/8; *16 folds SCALING=2.
            lgq = sg.tile([ER, NT4], F32)
            for t4 in range(NT4):
                nc.vector.reduce_sum(
                    lgq[:, t4 : t4 + 1], hps[t4][ER:128, :], axis=mybir.AxisListType.X
                )
            lg = sg.tile([ER, 1], F32)
            nc.vector.reduce_sum(lg[:], lgq[:], axis=mybir.AxisListType.X)
            nc.scalar.activation(
                lg[:], lg[:], mybir.ActivationFunctionType.Copy,
                scale=1.0 / (T * ROUTER_TEMP),
            )
            nc.vector.tensor_tensor(lg[:], lg[:], rb[:], mybir.AluOpType.add)
            mx = sg.tile([ER, 1], F32)
            nc.gpsimd.partition_all_reduce(
                mx[:], lg[:], channels=ER, reduce_op=bass_isa.ReduceOp.max
            )
            nc.vector.tensor_tensor(lg[:], lg[:], mx[:], mybir.AluOpType.subtract)
            nc.scalar.activation(lg[:], lg[:], mybir.ActivationFunctionType.Exp)
            sm = sg.tile([ER, 1], F32)
            nc.gpsimd.partition_all_reduce(
                sm[:], lg[:], channels=ER, reduce_op=bass_isa.ReduceOp.add
            )
            # w64 = exp/(8*S) = w/8; the 8*SCALING factor is pre-folded into
            # the host-side BTA rows.
            rcp = sg.tile([ER, 1], F32)
            nc.vector.reciprocal(rcp[:], sm[:])
            w64 = sg.tile([ER, 1], F32)
            nc.vector.tensor_tensor(w64[:], lg[:], rcp[:], mybir.AluOpType.mult)
            for t4 in range(NT4):
                nc.vector.tensor_tensor(
                    haug[0:ER, t4 * 512 : (t4 + 1) * 512],
                    hps[t4][0:ER, :],
                    w64[:].to_broadcast([ER, 512]),
                    mybir.AluOpType.mult,
                )

            def lora_and_evac(o, pso):
                for t4 in range(NT4):
                    nc.tensor.matmul(
                        pso[t4][:],
                        bta[:, o * 128 : (o + 1) * 128],
                        haug[:, t4 * 512 : (t4 + 1) * 512],
                        start=False,
                        stop=True,
                    )
                for t4 in range(NT4):
                    ev = evp.tile([128, 512], F32, tag="ev")
                    nc.vector.tensor_copy(ev[:], pso[t4][:])
                    nc.sync.dma_start(
                        out[o * 128 : (o + 1) * 128, t4 * 512 : (t4 + 1) * 512], ev[:]
                    )

            prev = (0, ps0)
            for o in range(1, OT):
                wt = load_w(o)
                pso = [
                    psp.tile([128, 512], F32, tag="ps", name=f"p{o}_{t}")
                    for t in range(NT4)
                ]
                for d in range(DT):
                    for t4 in range(NT4):
                        nc.tensor.matmul(
                            pso[t4][:],
                            wt[:, d, :],
                            xs[:, d, t4 * 512 : (t4 + 1) * 512],
                            start=(d == 0),
                            stop=False,
                        )
                lora_and_evac(*prev)
                prev = (o, pso)
            lora_and_evac(*prev)
    nc.compile()
    return nc


def _get_nc():
    if not _nc_cache:
        _nc_cache.append(build())
    return _nc_cache[0]


def _tile_dmajor(a_dT):
    """[D, C] (d-major rows) -> [128, DT*C] with d = dt*128 + p."""
    Dd, C = a_dT.shape
    return np.ascontiguousarray(
        a_dT.reshape(Dd // 128, 128, C).swapaxes(0, 1)
    ).reshape(128, (Dd // 128) * C)


def kernel(x, W_base, b_base, lora_A, lora_B, router_W, router_b):
    x = np.asarray(x, dtype=np.float32)
    W_base = np.asarray(W_base, dtype=np.float32)
    b_base = np.asarray(b_base, dtype=np.float32)
    lora_A = np.asarray(lora_A, dtype=np.float32)
    lora_B = np.asarray(lora_B, dtype=np.float32)
    router_W = np.asarray(router_W, dtype=np.float32)
    router_b = np.asarray(router_b, dtype=np.float32)

    B, S, D_ = x.shape
    O = W_base.shape[0]

    xs_list = []
    for b in range(B):
        xt = np.ascontiguousarray(x[b].astype(NPBF).T)  # [D, T]
        xs_list.append(_tile_dmajor(xt))

    wt_list = []
    for hh in range(2):
        Wh = np.ascontiguousarray(W_base[hh * O_SH : (hh + 1) * O_SH].astype(NPBF).T)
        # [D, O_SH] -> [OT, 128, DT*128]
        w4 = Wh.reshape(DT, 128, OT, 128).transpose(2, 1, 0, 3)
        wt_list.append(np.ascontiguousarray(w4).reshape(OT * 128, DT * 128))

    At = lora_A.reshape(ER, D_)  # [er, d]
    rw64 = np.repeat(router_W, R, axis=0)  # [er, d]
    arw_h = _tile_dmajor(
        np.ascontiguousarray(np.concatenate([At, rw64], axis=0).astype(NPBF).T)
    )

    bta_list = []
    for hh in range(2):
        osl = slice(hh * O_SH, (hh + 1) * O_SH)
        # 8*SCALING pre-folds the replicated-softmax 1/8 and the LoRA scaling,
        # so the device only computes w/8 = exp/(8*S).
        Bt = lora_B[:, osl, :].transpose(0, 2, 1).reshape(ER, O_SH) * (8.0 * SCALING)
        bta_list.append(
            np.ascontiguousarray(
                np.concatenate([Bt, b_base[osl][None, :]], axis=0).astype(NPBF)
            )
        )
    rb64 = np.ascontiguousarray(np.repeat(router_b, R).astype(np.float32))

    in_maps = []
    for c in range(8):
        b, hh = c // 2, c % 2
        in_maps.append(
            {
                "XS": xs_list[b],
                "WT": wt_list[hh],
                "ARW": arw_h,
                "BTA": bta_list[hh],
                "RB": rb64,
            }
        )

    global _last_in_maps
    _last_in_maps = in_maps
    nc = _get_nc()
    res = run_bass_kernel_spmd(nc, in_maps, core_ids=list(range(8)))
    out = np.empty((B, S, O), dtype=np.float32)
    for c in range(8):
        b, hh = c // 2, c % 2
        out[b, :, hh * O_SH : (hh + 1) * O_SH] = res.results[c]["out"].T
    return out
